# revision 5
# baseline (speedup 1.0000x reference)
"""Trainium2 Bass kernel for BertAlibiUnpadSelfAttention.

Problem shapes (hardcoded): B=2, S=2048, H=12, D=64, DIM=768.
Reference computation:
    qkv = hidden @ Wqkv_w.T + Wqkv_b            # (4096, 2304)
    pad via indices (a permutation -> pure row shuffle)
    q,k,v = split/reshape -> (b, h, s, d)
    scores = q @ k.T / sqrt(64) + bias          # bias dense (2,12,2048,2048)
    attn = softmax(scores) @ v -> (4096, 768), unpad via indices

Sharding: 24 (batch, head) pairs -> 3 per core across 8 cores. Each core
computes its own slice of the QKV projection (disjoint columns/rows -> no
redundant FLOPs) and full attention for its 3 heads.

Device kernel layout choices:
  - qT/kT computed in [d, s] layout directly (lhsT = W slices, rhs = hidden^T),
    which is exactly the layout the scores matmul wants.  1/sqrt(D) folded
    into Wq + bq on the host.
  - scores are computed TRANSPOSED: scoresT[sk, sq] tiles, so the softmax
    reduction (over sk) can be done by the PV matmul itself: V gets an
    appended ones-column, so PV produces [attnT ; sums] in one accumulation.
  - bias is pre-transposed per head on the host; VectorE adds it to the
    score PSUM; ScalarE applies exp (no max subtraction: logits ~ N(0,1),
    fp32 exp is exact-safe here).
  - Final normalize (divide by sums) + transpose back to [s, d] + V-bias add
    happen on the host (tiny: 3x65x2048 per core).
"""

import math
import numpy as np

B, S, H, D = 2, 2048, 12, 64
DIM = H * D            # 768
TOTAL = B * S          # 4096
HPC = 3                # heads per core
N_CORES = 8
KT = DIM // 128        # 6 k-tiles of 128
SQC = S // 512         # 4 free-dim chunks of 512
SKT = S // 128         # 16 sk tiles of 128

_CACHE = {}


def _build_nc(mm="f32r"):
    """Build + compile the per-core Bass module.

    mm selects matmul operand dtypes (all matmuls accumulate fp32 in PSUM):
      "f32"   - plain fp32 everywhere (4 cyc/row on PE, most accurate)
      "f32r"  - tf32 (fp32r) everywhere (1 cyc/row, ~1e-4 operand rounding)
      "mixed" - fp32 QKV projection, tf32 attention matmuls
    """
    from concourse import bacc, mybir, tile

    f32 = mybir.dt.float32
    proj_dt = mybir.dt.float32r if mm == "f32r" else f32
    att_dt = mybir.dt.float32r if mm in ("f32r", "mixed") else f32

    def mmap(ap):
        return ap

    nc = bacc.Bacc("TRN2", target_bir_lowering=False, debug=False)

    hT = nc.dram_tensor("hT", (DIM, S), proj_dt, kind="ExternalInput")
    wq = nc.dram_tensor("wq", (DIM, HPC * D), proj_dt, kind="ExternalInput")
    wk = nc.dram_tensor("wk", (DIM, HPC * D), proj_dt, kind="ExternalInput")
    wv = nc.dram_tensor("wv", (DIM, HPC * D), proj_dt, kind="ExternalInput")
    bq = nc.dram_tensor("bq", (HPC * D, 1), f32, kind="ExternalInput")
    bk = nc.dram_tensor("bk", (HPC * D, 1), f32, kind="ExternalInput")
    bias_t = nc.dram_tensor("bias_t", (HPC, S, S), f32, kind="ExternalInput")
    out = nc.dram_tensor("out", (HPC, D + 1, S), f32, kind="ExternalOutput")

    EXP = mybir.ActivationFunctionType.Exp

    with tile.TileContext(nc) as tc:
        with (
            tc.tile_pool(name="const", bufs=1) as constp,
            tc.tile_pool(name="bias", bufs=3) as biasp,
            tc.tile_pool(name="pt", bufs=2) as ptp,
            tc.tile_pool(name="ot", bufs=3) as otp,
        ):
            # ---- load persistent inputs ----
            ht = [constp.tile([128, S], proj_dt, tag=f"ht{i}", name=f"ht{i}") for i in range(KT)]
            for i in range(KT):
                nc.sync.dma_start(ht[i][:], hT[i * 128:(i + 1) * 128, :])
            wq_sb = [constp.tile([128, HPC * D], proj_dt, tag=f"wq{i}", name=f"wq{i}") for i in range(KT)]
            wk_sb = [constp.tile([128, HPC * D], proj_dt, tag=f"wk{i}", name=f"wk{i}") for i in range(KT)]
            wv_sb = [constp.tile([128, HPC * D], proj_dt, tag=f"wv{i}", name=f"wv{i}") for i in range(KT)]
            for i in range(KT):
                nc.sync.dma_start(wq_sb[i][:], wq[i * 128:(i + 1) * 128, :])
                nc.sync.dma_start(wk_sb[i][:], wk[i * 128:(i + 1) * 128, :])
                nc.sync.dma_start(wv_sb[i][:], wv[i * 128:(i + 1) * 128, :])
            bq_sb = constp.tile([128, 1], f32, tag="bq0")
            bq_sb2 = constp.tile([64, 1], f32, tag="bq1")
            bk_sb = constp.tile([128, 1], f32, tag="bk0")
            bk_sb2 = constp.tile([64, 1], f32, tag="bk1")
            nc.sync.dma_start(bq_sb[:], bq[0:128, :])
            nc.sync.dma_start(bq_sb2[:], bq[128:192, :])
            nc.sync.dma_start(bk_sb[:], bk[0:128, :])
            nc.sync.dma_start(bk_sb2[:], bk[128:192, :])

            # Q/K in [d, s] layout: heads 0,1 in tile0 (partitions 0-63 /
            # 64-127), head 2 in tile1 (partitions 0-63).  Same base
            # partition for q_j and k_j so the scores matmul operands align.
            q0 = constp.tile([128, S], att_dt, tag="q0")
            q1 = constp.tile([64, S], att_dt, tag="q1")
            k0 = constp.tile([128, S], att_dt, tag="k0")
            k1 = constp.tile([64, S], att_dt, tag="k1")
            # V' per head: [sk, 65] blocks stacked along free dim; col 64
            # stays 1.0 so PV also produces the softmax row-sums.
            vp = [constp.tile([128, SKT * (D + 1)], att_dt, tag=f"vp{j}", name=f"vp{j}")
                  for j in range(HPC)]
            for j in range(HPC):
                nc.vector.memset(vp[j][:].bitcast(f32), 1.0)

            # ---- phase 1a: qT / kT projection (+ bias, per-partition) ----
            with tc.tile_pool(name="psA", bufs=2, space="PSUM") as psA:
                for (dst, wsb, bsb, col0, m) in (
                    (q0, wq_sb, bq_sb, 0, 128),
                    (q1, wq_sb, bq_sb2, 128, 64),
                    (k0, wk_sb, bk_sb, 0, 128),
                    (k1, wk_sb, bk_sb2, 128, 64),
                ):
                    for c in range(SQC):
                        ps = psA.tile([m, 512], f32, tag=f"psA{m}", name=f"psA{m}")
                        for i in range(KT):
                            nc.tensor.matmul(
                                ps[:],
                                mmap(wsb[i][:, col0:col0 + m]),
                                mmap(ht[i][:, c * 512:(c + 1) * 512]),
                                start=(i == 0), stop=(i == KT - 1),
                            )
                        nc.vector.tensor_scalar_add(
                            dst[:, c * 512:(c + 1) * 512], ps[:], bsb[:])

                # ---- phase 1b: V in natural [s, d] layout ----
                for st in range(SKT):
                    psv = psA.tile([128, HPC * D], f32, tag="psV", name="psV")
                    for i in range(KT):
                        nc.tensor.matmul(
                            psv[:],
                            mmap(ht[i][:, st * 128:(st + 1) * 128]),
                            mmap(wv_sb[i][:]),
                            start=(i == 0), stop=(i == KT - 1),
                        )
                    for j in range(HPC):
                        nc.vector.tensor_copy(
                            vp[j][:, st * (D + 1):st * (D + 1) + D],
                            psv[:, j * D:(j + 1) * D])

            # ---- phase 2: attention per head ----
            qk_slices = (  # (q_ap, k_ap) per head, matching base partitions
                (q0[0:64, :], k0[0:64, :]),
                (q0[64:128, :], k0[64:128, :]),
                (q1[:, :], k1[:, :]),
            )
            with (
                tc.tile_pool(name="ps", bufs=3, space="PSUM") as psp,
                tc.tile_pool(name="po", bufs=4, space="PSUM") as pop,
            ):
                for j in range(HPC):
                    qap, kap = qk_slices[j]
                    po = [pop.tile([D + 1, 512], f32, tag="po", name=f"po{j}_{_c}") for _c in range(SQC)]
                    for st in range(SKT):
                        bt = biasp.tile([128, S], f32, name="bt")
                        nc.sync.dma_start(
                            bt[:], bias_t[j, st * 128:(st + 1) * 128, :])
                        pt = ptp.tile([128, S], att_dt, name="pt")
                        for c in range(SQC):
                            ps = psp.tile([128, 512], f32, name="ps")
                            nc.tensor.matmul(
                                ps[:],
                                mmap(kap[:, st * 128:(st + 1) * 128]),
                                mmap(qap[:, c * 512:(c + 1) * 512]),
                                start=True, stop=True,
                            )
                            nc.vector.tensor_add(
                                pt[:, c * 512:(c + 1) * 512],
                                ps[:], bt[:, c * 512:(c + 1) * 512])
                        nc.scalar.activation(pt[:], pt[:], EXP)
                        for c in range(SQC):
                            nc.tensor.matmul(
                                po[c][:],
                                mmap(vp[j][:, st * (D + 1):(st + 1) * (D + 1)]),
                                mmap(pt[:, c * 512:(c + 1) * 512]),
                                start=(st == 0), stop=(st == SKT - 1),
                            )
                    for c in range(SQC):
                        ot = otp.tile([D + 1, 512], f32, name="ot")
                        nc.scalar.copy(ot[:], po[c][:])
                        nc.sync.dma_start(
                            out[j, :, c * 512:(c + 1) * 512], ot[:])

    nc.compile()
    return nc


def _get_nc(mm="f32r"):
    if mm not in _CACHE:
        _CACHE[mm] = _build_nc(mm)
    return _CACHE[mm]


def _make_in_maps(hidden_states, Wqkv_w, Wqkv_b, bias, indices):
    hidden_states = np.asarray(hidden_states, dtype=np.float32)
    Wqkv_w = np.asarray(Wqkv_w, dtype=np.float32)
    Wqkv_b = np.asarray(Wqkv_b, dtype=np.float32)
    bias = np.asarray(bias, dtype=np.float32)
    indices = np.asarray(indices, dtype=np.int64)

    scale = 1.0 / math.sqrt(D)
    padded = np.zeros((TOTAL, DIM), dtype=np.float32)
    padded[indices] = hidden_states

    Wq, Wk, Wv = Wqkv_w[0:DIM], Wqkv_w[DIM:2 * DIM], Wqkv_w[2 * DIM:3 * DIM]
    bq_full = Wqkv_b[0:DIM] * scale
    bk_full = Wqkv_b[DIM:2 * DIM]

    in_maps = []
    for c in range(N_CORES):
        b = c // 4
        h0 = (c % 4) * HPC
        r = slice(h0 * D, (h0 + HPC) * D)
        in_maps.append({
            "hT": np.ascontiguousarray(padded[b * S:(b + 1) * S].T),
            "wq": np.ascontiguousarray(Wq[r].T) * np.float32(scale),
            "wk": np.ascontiguousarray(Wk[r].T),
            "wv": np.ascontiguousarray(Wv[r].T),
            "bq": np.ascontiguousarray(bq_full[r].reshape(HPC * D, 1)),
            "bk": np.ascontiguousarray(bk_full[r].reshape(HPC * D, 1)),
            "bias_t": np.ascontiguousarray(
                bias[b, h0:h0 + HPC].transpose(0, 2, 1)),
        })
    return in_maps


def _assemble(results, Wqkv_b, indices):
    Wqkv_b = np.asarray(Wqkv_b, dtype=np.float32)
    indices = np.asarray(indices, dtype=np.int64)
    bv = Wqkv_b[2 * DIM:3 * DIM]
    out_full = np.empty((TOTAL, DIM), dtype=np.float32)
    for c in range(N_CORES):
        b = c // 4
        h0 = (c % 4) * HPC
        o = np.asarray(results[c]["out"], dtype=np.float32)  # (3, 65, 2048)
        for j in range(HPC):
            h = h0 + j
            att = (o[j, :D] / o[j, D]).T + bv[h * D:(h + 1) * D]
            out_full[b * S:(b + 1) * S, h * D:(h + 1) * D] = att
    return out_full[indices]


def kernel(hidden_states, Wqkv_w, Wqkv_b, bias, slopes, cu_seqlens, indices,
           attn_mask, max_seqlen, **_unused):
    from concourse.bass_utils import run_bass_kernel_spmd

    nc = _get_nc()
    in_maps = _make_in_maps(hidden_states, Wqkv_w, Wqkv_b, bias, indices)
    res = run_bass_kernel_spmd(nc, in_maps, list(range(N_CORES)))
    return _assemble(res.results, Wqkv_b, indices)
